# revision 14
# baseline (speedup 1.0000x reference)
"""Trainium2 Bass kernel for nn_GaussianRecurrent.

Math: the reference scans t=0..T-1 with
    lkd += sum_d[-0.5*log(2*pi*var_t) - (z_t-mu_t)^2/(2*var_t)]
    dd_t = c/(v + c*t);  mu <- (1-dd)mu + dd z;  var <- (1-dd)var + (v-c)dd
var_t and dd_t are data-independent. With uniform per-feature params
(r = v/c = 1/sigmoid(corr)):
    mu_t  = ((r-1)*mu0 + sum_{s<t} z_s) / (r+t-1)  = u_t*(M0 + C_t)
    var_t = ((r-1)*v + (v-c)*t) / (r+t-1)
so lkd = const - sum_{t,d} g_t*(z - u_t*C_t)^2,  g_t = 1/(2 var_t).

Device kernel (8 cores, T time-sharded): per [128t x 512d] tile,
  - matmul with a shared strict-upper-triangular lhsT -> exclusive prefix C
  - matmul with per-tile [128,2] lhsT (b_j | ones) -> S2_j and column totals
  - DVE: diff = z - u*C (per-partition scalar u)
  - ScalarE: Square(sqrt(g)*diff) with accum_out -> per-partition tile sums
Cross-tile/core prefix offsets V_j are folded in exactly on the host:
  sum g (diff - u V)^2 = Q - 2 V.S2 + S3 |V|^2  (per tile, f64 combine).
"""
import numpy as np

T = 65536
D = 512
NCORES = 8
TPC = T // NCORES          # 8192 timesteps per core
TILE = 128
NTILES = TPC // TILE       # 64 tiles per core

_cache = {}


def _build_program():
    import concourse.bass as bass
    import concourse.tile as tile
    import concourse.mybir as mybir
    from concourse import bacc

    f32 = mybir.dt.float32
    f32r = mybir.dt.float32r

    nc = bacc.Bacc("TRN2", target_bir_lowering=False, debug=False)
    z_d = nc.dram_tensor("z", [TPC, D], f32, kind="ExternalInput")
    su_d = nc.dram_tensor("su", [128, 128], f32, kind="ExternalInput")
    btl_d = nc.dram_tensor("btl", [128, NTILES, 128], f32, kind="ExternalInput")
    negu_d = nc.dram_tensor("negu", [128, NTILES], f32, kind="ExternalInput")
    sg_d = nc.dram_tensor("sg", [128, NTILES], f32, kind="ExternalInput")
    bt_d = nc.dram_tensor("bt_out", [128, D], f32, kind="ExternalOutput")
    q_d = nc.dram_tensor("q_out", [128, NTILES], f32, kind="ExternalOutput")

    zt = z_d.ap().rearrange("(n p) d -> n p d", p=TILE)

    with tile.TileContext(nc) as tc:
        with (
            tc.tile_pool(name="const", bufs=1) as cpool,
            tc.tile_pool(name="zp", bufs=6) as zp,
            tc.tile_pool(name="dp", bufs=4) as dp,
            tc.tile_pool(name="qp", bufs=1) as qp,
            tc.tile_pool(name="psC", bufs=3, space=bass.MemorySpace.PSUM) as psC,
            tc.tile_pool(name="psB", bufs=1, space=bass.MemorySpace.PSUM) as psB,
            tc.tile_pool(name="psS", bufs=2, space=bass.MemorySpace.PSUM) as psS,
        ):
            su = cpool.tile([128, 128], f32r)
            btl = cpool.tile([128, NTILES, 128], f32r)
            negu = cpool.tile([128, NTILES], f32)
            sg = cpool.tile([128, NTILES], f32)
            qbuf = qp.tile([128, NTILES], f32)
            btsb = qp.tile([128, D], f32)
            btPS = psB.tile([128, D], f32)
            nc.sync.dma_start(su[:], su_d.ap().bitcast(f32r))
            nc.scalar.dma_start(btl[:], btl_d.ap().bitcast(f32r))
            nc.sync.dma_start(negu[:], negu_d.ap())
            nc.sync.dma_start(sg[:], sg_d.ap())

            for j in range(NTILES):
                z = zp.tile([128, D], f32r)
                dma_eng = (nc.sync, nc.gpsimd, nc.scalar)[j % 3]
                dma_eng.dma_start(z[:], zt[j].bitcast(f32r))

                C = psC.tile([128, D], f32)
                nc.tensor.matmul(C[:], su[:], z[:])
                # tile j's [b_j|ones] live in cols 2j,2j+1 of its lhsT;
                # the other 126 columns are zero, so all 64 tiles accumulate
                # disjointly into the persistent btPS bank.
                nc.tensor.matmul(
                    btPS[:], btl[:, j, :], z[:],
                    start=(j == 0), stop=(j == NTILES - 1),
                )

                diff = dp.tile([128, D], f32)
                nc.vector.scalar_tensor_tensor(
                    diff[:], C[:], negu[:, j : j + 1], z[:].bitcast(f32),
                    mybir.AluOpType.mult, mybir.AluOpType.add,
                )
                sq = psS.tile([128, D], f32)
                nc.scalar.activation(
                    sq[:], diff[:], mybir.ActivationFunctionType.Square,
                    bias=0.0, scale=sg[:, j : j + 1], accum_out=qbuf[:, j : j + 1],
                )

            nc.scalar.copy(btsb[:], btPS[:])
            nc.sync.dma_start(bt_d.ap(), btsb[:])
            nc.sync.dma_start(q_d.ap(), qbuf[:])

    nc.compile()
    return nc


def _host_scan(z_rest, var_vbl, corr_vbl, prior_mu):
    z = z_rest.astype(np.float64)
    v = np.square(np.log1p(np.exp(var_vbl.astype(np.float64))))
    c = v / (1.0 + np.exp(-corr_vbl.astype(np.float64)))
    mu = prior_mu.astype(np.float64).copy()
    var = v.copy()
    lkd = 0.0
    for t in range(z.shape[0]):
        lkd += np.sum(-0.5 * np.log(2 * np.pi * var) - (z[t] - mu) ** 2 / (2 * var))
        dd = c / (v + c * t)
        mu = (1 - dd) * mu + z[t] * dd
        var = (1 - dd) * var + (v - c) * dd
    return np.float32(lkd)


def kernel(z_rest, var_vbl, corr_vbl, prior_mu):
    z_rest = np.ascontiguousarray(np.asarray(z_rest, dtype=np.float32))
    var_vbl = np.asarray(var_vbl, dtype=np.float32)
    corr_vbl = np.asarray(corr_vbl, dtype=np.float32)
    prior_mu = np.asarray(prior_mu, dtype=np.float32)

    if not (np.all(var_vbl == var_vbl[0]) and np.all(corr_vbl == corr_vbl[0])):
        return _host_scan(z_rest, var_vbl, corr_vbl, prior_mu)

    # coefficients (f64)
    v = float(np.square(np.log1p(np.exp(np.float64(var_vbl[0])))))
    gamma = float(1.0 / (1.0 + np.exp(-np.float64(corr_vbl[0]))))
    c = gamma * v
    r = 1.0 / gamma
    if not np.isfinite(r) or r <= 1.0 + 1e-6 or v <= 0:
        return _host_scan(z_rest, var_vbl, corr_vbl, prior_mu)

    t = np.arange(T, dtype=np.float64)
    u = 1.0 / (r + t - 1.0)
    var_t = ((r - 1.0) * v + (v - c) * t) / (r + t - 1.0)
    g = 1.0 / (2.0 * var_t)
    w = g * u
    sg_all = np.sqrt(g)
    const = -0.5 * D * float(np.sum(np.log(2 * np.pi * var_t)))
    GTILES = T // TILE
    wu = (w * u).reshape(GTILES, TILE)
    S3 = wu.sum(axis=1)  # = sum g u^2 per tile
    # b_j[s] = w_s - sum_{t>s in tile} w_t u_t
    wt = w.reshape(GTILES, TILE)
    b = wt - (np.cumsum(wu[:, ::-1], axis=1)[:, ::-1] - wu)

    # per-core device inputs
    su_np = np.triu(np.ones((128, 128), dtype=np.float32), k=1)
    in_maps = []
    for k in range(NCORES):
        sl = slice(k * GTILES // NCORES, (k + 1) * GTILES // NCORES)
        btl_np = np.zeros((128, NTILES, 128), dtype=np.float32)
        bT = b[sl].T.astype(np.float32)  # [128, NTILES]
        for jj in range(NTILES):
            btl_np[:, jj, 2 * jj] = bT[:, jj]
            btl_np[:, jj, 2 * jj + 1] = 1.0
        negu_np = (-u.reshape(GTILES, TILE)[sl].T).astype(np.float32)
        sg_np = sg_all.reshape(GTILES, TILE)[sl].T.astype(np.float32)
        in_maps.append({
            "z": z_rest[k * TPC : (k + 1) * TPC],
            "su": su_np,
            "btl": btl_np,
            "negu": negu_np,
            "sg": sg_np,
        })

    from concourse.bass_utils import run_bass_kernel_spmd

    if "nc" not in _cache:
        _cache["nc"] = _build_program()
    import os
    tmpdir = os.environ.get("BASS_KERNEL_TMPDIR") or None
    if tmpdir:
        os.makedirs(tmpdir, exist_ok=True)
    res = run_bass_kernel_spmd(
        _cache["nc"], in_maps, list(range(NCORES)), tmpdir=tmpdir
    )
    _cache["last_results"] = res

    # host combine in f64
    M0 = (r - 1.0) * prior_mu.astype(np.float64)
    lkd = const
    V = M0.copy()
    for k in range(NCORES):
        bt = res.results[k]["bt_out"].astype(np.float64)   # [128, D]
        q = res.results[k]["q_out"].astype(np.float64)     # [128, NTILES]
        qs = q.sum(axis=0)                                  # Q_j per tile
        for jj in range(NTILES):
            gj = k * NTILES + jj
            S2 = bt[2 * jj]
            tot = bt[2 * jj + 1]
            lkd += -qs[jj] + 2.0 * np.dot(V, S2) - S3[gj] * np.dot(V, V)
            V += tot
    return np.float32(lkd)


if __name__ == "__main__":
    import sys
    sys.path.insert(0, "/root/problem")
    from reference import setup_inputs
    inputs = {k: np.asarray(v) for k, v in setup_inputs().items()}
    out = kernel(**inputs)
    print("kernel lkd:", out)


# revision 16
# speedup vs baseline: 1.2043x; 1.2043x over previous
"""Trainium2 Bass kernel for nn_GaussianRecurrent.

Math: the reference scans t=0..T-1 with
    lkd += sum_d[-0.5*log(2*pi*var_t) - (z_t-mu_t)^2/(2*var_t)]
    dd_t = c/(v + c*t);  mu <- (1-dd)mu + dd z;  var <- (1-dd)var + (v-c)dd
var_t and dd_t are data-independent. With uniform per-feature params
(r = v/c = 1/sigmoid(corr)):
    mu_t  = ((r-1)*mu0 + sum_{s<t} z_s) / (r+t-1)  = u_t*(M0 + C_t)
    var_t = ((r-1)*v + (v-c)*t) / (r+t-1)
so lkd = const - sum_{t,d} g_t*(z - u_t*C_t)^2,  g_t = 1/(2 var_t).

Device kernel (8 cores, T time-sharded): per [128t x 512d] tile,
  - matmul with a shared strict-upper-triangular lhsT -> exclusive prefix C
  - matmul with per-tile [128,2] lhsT (b_j | ones) -> S2_j and column totals
  - DVE: diff = z - u*C (per-partition scalar u)
  - ScalarE: Square(sqrt(g)*diff) with accum_out -> per-partition tile sums
Cross-tile/core prefix offsets V_j are folded in exactly on the host:
  sum g (diff - u V)^2 = Q - 2 V.S2 + S3 |V|^2  (per tile, f64 combine).
"""
import numpy as np

T = 65536
D = 512
NCORES = 8
TPC = T // NCORES          # 8192 timesteps per core
TILE = 128
NTILES = TPC // TILE       # 64 tiles per core

_cache = {}


def _build_program():
    import concourse.bass as bass
    import concourse.tile as tile
    import concourse.mybir as mybir
    from concourse import bacc

    f32 = mybir.dt.float32
    f32r = mybir.dt.float32r

    nc = bacc.Bacc("TRN2", target_bir_lowering=False, debug=False)
    z_d = nc.dram_tensor("z", [TPC, D], f32, kind="ExternalInput")
    su_d = nc.dram_tensor("su", [128, 128], f32, kind="ExternalInput")
    btl_d = nc.dram_tensor("btl", [128, NTILES, 128], f32, kind="ExternalInput")
    negu_d = nc.dram_tensor("negu", [128, NTILES], f32, kind="ExternalInput")
    sg_d = nc.dram_tensor("sg", [128, NTILES], f32, kind="ExternalInput")
    gf_d = nc.dram_tensor("gf", [128, NTILES], f32, kind="ExternalInput")
    bt_d = nc.dram_tensor("bt_out", [128, D], f32, kind="ExternalOutput")
    q_d = nc.dram_tensor("q_out", [128, NTILES], f32, kind="ExternalOutput")

    zt = z_d.ap().rearrange("(n p) d -> n p d", p=TILE)

    with tile.TileContext(nc) as tc:
        with (
            tc.tile_pool(name="const", bufs=1) as cpool,
            tc.tile_pool(name="zp", bufs=6) as zp,
            tc.tile_pool(name="dp", bufs=4) as dp,
            tc.tile_pool(name="qp", bufs=1) as qp,
            tc.tile_pool(name="psC", bufs=3, space=bass.MemorySpace.PSUM) as psC,
            tc.tile_pool(name="psB", bufs=1, space=bass.MemorySpace.PSUM) as psB,
            tc.tile_pool(name="psS", bufs=2, space=bass.MemorySpace.PSUM) as psS,
        ):
            su = cpool.tile([128, 128], f32r)
            btl = cpool.tile([128, NTILES, 128], f32r)
            negu = cpool.tile([128, NTILES], f32)
            sg = cpool.tile([128, NTILES], f32)
            gf = cpool.tile([128, NTILES], f32)
            qbuf = qp.tile([128, NTILES], f32)
            btsb = qp.tile([128, D], f32)
            btPS = psB.tile([128, D], f32)
            nc.sync.dma_start(su[:], su_d.ap().bitcast(f32r))
            nc.sync.dma_start(btl[:], btl_d.ap().bitcast(f32r))
            nc.sync.dma_start(negu[:], negu_d.ap())
            nc.sync.dma_start(sg[:], sg_d.ap())
            nc.gpsimd.dma_start(gf[:], gf_d.ap())

            for j in range(NTILES):
                z = zp.tile([128, D], f32r)
                dma_eng = nc.sync if j % 2 == 0 else nc.gpsimd
                dma_eng.dma_start(z[:], zt[j].bitcast(f32r))

                C = psC.tile([128, D], f32)
                nc.tensor.matmul(C[:], su[:], z[:])
                # tile j's [b_j|ones] live in cols 2j,2j+1 of its lhsT;
                # the other 126 columns are zero, so all 64 tiles accumulate
                # disjointly into the persistent btPS bank.
                nc.tensor.matmul(
                    btPS[:], btl[:, j, :], z[:],
                    start=(j == 0), stop=(j == NTILES - 1),
                )

                diff = dp.tile([128, D], f32)
                nc.vector.scalar_tensor_tensor(
                    diff[:], C[:], negu[:, j : j + 1], z[:].bitcast(f32),
                    mybir.AluOpType.mult, mybir.AluOpType.add,
                )
                sq = psS.tile([128, D], f32)
                if j % 4 == 3:
                    # DVE path: (diff*g)*diff = g*diff^2, accum = row sums
                    nc.vector.scalar_tensor_tensor(
                        sq[:], diff[:], gf[:, j : j + 1], diff[:],
                        mybir.AluOpType.mult, mybir.AluOpType.mult,
                        accum_out=qbuf[:, j : j + 1],
                    )
                else:
                    nc.scalar.activation(
                        sq[:], diff[:], mybir.ActivationFunctionType.Square,
                        bias=0.0, scale=sg[:, j : j + 1], accum_out=qbuf[:, j : j + 1],
                    )

            nc.scalar.copy(btsb[:], btPS[:])
            nc.sync.dma_start(bt_d.ap(), btsb[:])
            nc.sync.dma_start(q_d.ap(), qbuf[:])

    nc.compile()
    return nc


def _host_scan(z_rest, var_vbl, corr_vbl, prior_mu):
    z = z_rest.astype(np.float64)
    v = np.square(np.log1p(np.exp(var_vbl.astype(np.float64))))
    c = v / (1.0 + np.exp(-corr_vbl.astype(np.float64)))
    mu = prior_mu.astype(np.float64).copy()
    var = v.copy()
    lkd = 0.0
    for t in range(z.shape[0]):
        lkd += np.sum(-0.5 * np.log(2 * np.pi * var) - (z[t] - mu) ** 2 / (2 * var))
        dd = c / (v + c * t)
        mu = (1 - dd) * mu + z[t] * dd
        var = (1 - dd) * var + (v - c) * dd
    return np.float32(lkd)


def kernel(z_rest, var_vbl, corr_vbl, prior_mu):
    z_rest = np.ascontiguousarray(np.asarray(z_rest, dtype=np.float32))
    var_vbl = np.asarray(var_vbl, dtype=np.float32)
    corr_vbl = np.asarray(corr_vbl, dtype=np.float32)
    prior_mu = np.asarray(prior_mu, dtype=np.float32)

    if not (np.all(var_vbl == var_vbl[0]) and np.all(corr_vbl == corr_vbl[0])):
        return _host_scan(z_rest, var_vbl, corr_vbl, prior_mu)

    # coefficients (f64)
    v = float(np.square(np.log1p(np.exp(np.float64(var_vbl[0])))))
    gamma = float(1.0 / (1.0 + np.exp(-np.float64(corr_vbl[0]))))
    c = gamma * v
    r = 1.0 / gamma
    if not np.isfinite(r) or r <= 1.0 + 1e-6 or v <= 0:
        return _host_scan(z_rest, var_vbl, corr_vbl, prior_mu)

    t = np.arange(T, dtype=np.float64)
    u = 1.0 / (r + t - 1.0)
    var_t = ((r - 1.0) * v + (v - c) * t) / (r + t - 1.0)
    g = 1.0 / (2.0 * var_t)
    w = g * u
    sg_all = np.sqrt(g)
    const = -0.5 * D * float(np.sum(np.log(2 * np.pi * var_t)))
    GTILES = T // TILE
    wu = (w * u).reshape(GTILES, TILE)
    S3 = wu.sum(axis=1)  # = sum g u^2 per tile
    # b_j[s] = w_s - sum_{t>s in tile} w_t u_t
    wt = w.reshape(GTILES, TILE)
    b = wt - (np.cumsum(wu[:, ::-1], axis=1)[:, ::-1] - wu)

    # per-core device inputs
    su_np = np.triu(np.ones((128, 128), dtype=np.float32), k=1)
    in_maps = []
    for k in range(NCORES):
        sl = slice(k * GTILES // NCORES, (k + 1) * GTILES // NCORES)
        btl_np = np.zeros((128, NTILES, 128), dtype=np.float32)
        bT = b[sl].T.astype(np.float32)  # [128, NTILES]
        for jj in range(NTILES):
            btl_np[:, jj, 2 * jj] = bT[:, jj]
            btl_np[:, jj, 2 * jj + 1] = 1.0
        negu_np = (-u.reshape(GTILES, TILE)[sl].T).astype(np.float32)
        sg_np = sg_all.reshape(GTILES, TILE)[sl].T.astype(np.float32)
        in_maps.append({
            "z": z_rest[k * TPC : (k + 1) * TPC],
            "su": su_np,
            "btl": btl_np,
            "negu": negu_np,
            "sg": sg_np,
            "gf": (sg_np.astype(np.float64) ** 2).astype(np.float32),
        })

    from concourse.bass_utils import run_bass_kernel_spmd

    if "nc" not in _cache:
        _cache["nc"] = _build_program()
    import os
    tmpdir = os.environ.get("BASS_KERNEL_TMPDIR") or None
    if tmpdir:
        os.makedirs(tmpdir, exist_ok=True)
    res = run_bass_kernel_spmd(
        _cache["nc"], in_maps, list(range(NCORES)), tmpdir=tmpdir
    )
    _cache["last_results"] = res

    # host combine in f64
    M0 = (r - 1.0) * prior_mu.astype(np.float64)
    lkd = const
    V = M0.copy()
    for k in range(NCORES):
        bt = res.results[k]["bt_out"].astype(np.float64)   # [128, D]
        q = res.results[k]["q_out"].astype(np.float64)     # [128, NTILES]
        qs = q.sum(axis=0)                                  # Q_j per tile
        for jj in range(NTILES):
            gj = k * NTILES + jj
            S2 = bt[2 * jj]
            tot = bt[2 * jj + 1]
            lkd += -qs[jj] + 2.0 * np.dot(V, S2) - S3[gj] * np.dot(V, V)
            V += tot
    return np.float32(lkd)


if __name__ == "__main__":
    import sys
    sys.path.insert(0, "/root/problem")
    from reference import setup_inputs
    inputs = {k: np.asarray(v) for k, v in setup_inputs().items()}
    out = kernel(**inputs)
    print("kernel lkd:", out)
